# revision 9
# baseline (speedup 1.0000x reference)
"""Trainium2 Bass kernel for nn_Attention_47545287967487.

Causal multi-head attention (B=2, S=2048, D=1024, H=16, DH=64) with QK
RMS-norm, distributed over 8 NeuronCores via head tensor-parallelism:
each core owns 2 heads (a 128-column slice of Wq/Wk/Wv and the matching
128-row slice of Wo), computes its partial output projection, and a
ReduceScatter produces each core's 512-row slice of the final output.

Numerics: projections and the output matmul run in float32r (~1e-4),
attention internals (QK^T, softmax, PV) run in bf16. Scores are bounded
(|q.k|/8 <= 8 after RMS-norm) so softmax skips the max-subtraction pass.

kernel(**inputs) takes the FULL unsharded inputs and returns the FULL
[2, 2048, 1024] float32 output.
"""

import numpy as np

import concourse.bacc as bacc
import concourse.mybir as mybir
from concourse import tile
from concourse.bass_utils import run_bass_kernel_spmd

import ml_dtypes

# All ACT functions this kernel uses (Square, Ln, Exp, Copy) live in the
# 'natural_log_exp_and_others' table. The default table chooser picks the
# first table containing each function, which thrashes between the exp and
# ln tables (~1.3us per reload, dozens of reloads). Pin the chooser to the
# one table that covers everything by emptying the others (positions are
# preserved so act_func_set_id still indexes act_info.json correctly).
_PINNED_ACT_TABLE = "natural_log_exp_and_others"
_orig_get_act_tables = bacc.get_activation_tables


def _pinned_act_tables(arch):
    tables = _orig_get_act_tables(arch)
    return {
        name: (funcs if name == _PINNED_ACT_TABLE else set())
        for name, funcs in tables.items()
    }


bacc.get_activation_tables = _pinned_act_tables

BF16 = ml_dtypes.bfloat16

# Problem shape (hardcoded per the harness contract).
B, S, D, DH = 2, 2048, 1024, 64
H = D // DH
N_CORES = 8
HEADS_PER_CORE = H // N_CORES          # 2
DC = HEADS_PER_CORE * DH               # 128 feature columns per core
EPS = 1e-6
SCALE = 1.0 / (DH ** 0.5)              # 1/8

SCHUNK = 512                            # s-chunk width
TT = 128                                # t-tile width
KT = D // 128                           # 8 contraction tiles
NCH = S // SCHUNK                       # 4 s-chunks per batch
ROWS = B * S                            # 4096
ROWS_PER_CORE = ROWS // N_CORES         # 512

F32 = mybir.dt.float32
F32R = mybir.dt.float32r
BF = mybir.dt.bfloat16


def build_nc():
    nc = bacc.Bacc("TRN2", target_bir_lowering=False)

    xt_d = nc.dram_tensor("xt", [D, ROWS], F32R, kind="ExternalInput")
    wq_d = nc.dram_tensor("wq", [D, DC], F32R, kind="ExternalInput")
    wk_d = nc.dram_tensor("wk", [D, DC], F32R, kind="ExternalInput")
    wv_d = nc.dram_tensor("wv", [D, DC], F32R, kind="ExternalInput")
    wo_d = nc.dram_tensor("wo", [DC, D], F32R, kind="ExternalInput")
    mask_d = nc.dram_tensor("mask0", [TT, SCHUNK], BF, kind="ExternalInput")
    ident_d = nc.dram_tensor("ident", [128, 128], BF, kind="ExternalInput")
    sel2_d = nc.dram_tensor("sel2", [128, 2], F32R, kind="ExternalInput")
    sel2t_d = nc.dram_tensor("sel2t", [2, 128], F32R, kind="ExternalInput")
    ones64_d = nc.dram_tensor("ones64", [1, 64], F32R, kind="ExternalInput")
    out_d = nc.dram_tensor("out", [ROWS_PER_CORE, D], F32, kind="ExternalOutput")

    from contextlib import ExitStack
    with tile.TileContext(nc) as tc:
        with ExitStack() as ctx:
            consts = ctx.enter_context(tc.tile_pool(name="consts", bufs=1))
            wpool = ctx.enter_context(tc.tile_pool(name="wpool", bufs=1))
            persist = ctx.enter_context(tc.tile_pool(name="persist", bufs=1))
            xcp = ctx.enter_context(tc.tile_pool(name="xc", bufs=16))
            sqp = ctx.enter_context(tc.tile_pool(name="sqp", bufs=2))
            stdp = ctx.enter_context(tc.tile_pool(name="stdp", bufs=4))
            vtp = ctx.enter_context(tc.tile_pool(name="vtp", bufs=2))
            vaugp = ctx.enter_context(tc.tile_pool(name="vaugp", bufs=40))
            pp = ctx.enter_context(tc.tile_pool(name="pp", bufs=4))
            zbp = ctx.enter_context(tc.tile_pool(name="zbp", bufs=2))
            rcp = ctx.enter_context(tc.tile_pool(name="rcp", bufs=2))
            attallp = ctx.enter_context(tc.tile_pool(name="attall", bufs=2))
            outsbp = ctx.enter_context(tc.tile_pool(name="outsb", bufs=4))
            ps_acc = ctx.enter_context(tc.tile_pool(name="ps_acc", bufs=3, space="PSUM"))
            ps_pt = ctx.enter_context(tc.tile_pool(name="ps_pt", bufs=2, space="PSUM"))
            ps_att = ctx.enter_context(tc.tile_pool(name="ps_att", bufs=2, space="PSUM"))
            ps_tiny = ctx.enter_context(tc.tile_pool(name="ps_tiny", bufs=1, space="PSUM"))
            dram = ctx.enter_context(tc.tile_pool(name="dram", bufs=1, space="DRAM"))

            # ---- constants / weights into SBUF ----
            mask_sb = consts.tile([TT, SCHUNK], BF, name="mask_sb")
            nc.sync.dma_start(mask_sb[:], mask_d[:])
            ident_sb = consts.tile([128, 128], BF, name="ident_sb")
            nc.sync.dma_start(ident_sb[:], ident_d[:])
            sel2_sb = consts.tile([128, 2], F32R, name="sel2_sb")
            nc.sync.dma_start(sel2_sb[:], sel2_d[:])
            sel2t_sb = consts.tile([2, 128], F32R, name="sel2t_sb")
            nc.sync.dma_start(sel2t_sb[:], sel2t_d[:])
            ones64_sb = consts.tile([1, 64], F32R, name="ones64_sb")
            nc.sync.dma_start(ones64_sb[:], ones64_d[:])
            eps_sb = consts.tile([128, 1], F32, name="eps_sb")
            nc.vector.memset(eps_sb[:], EPS)

            w_sb = {}
            for wname, wd in (("q", wq_d), ("k", wk_d), ("v", wv_d)):
                for k in range(KT):
                    t = wpool.tile([128, DC], F32R, name=f"w{wname}{k}")
                    nc.sync.dma_start(t[:], wd[k * 128:(k + 1) * 128, :])
                    w_sb[(wname, k)] = t
            wo_sb = wpool.tile([DC, D], F32R, name="wo_sb")
            nc.sync.dma_start(wo_sb[:], wo_d[:])

            partial = dram.tile([ROWS, D], F32, name="partial")
            rs_out = dram.tile([ROWS_PER_CORE, D], F32, name="rs_out")

            # persistent normalized q/k (feature-major, bf16) per batch
            qt_sb = [persist.tile([DC, S], BF, name=f"qt_b{b}") for b in range(B)]
            kt_sb = [persist.tile([DC, S], BF, name=f"kt_b{b}") for b in range(B)]
            vaug = {}  # (b, j) -> [128, 2*(DH+1)] bf16

            def rms_norm_store(acc_psum, dst, b, i):
                """Normalize feature-major [128, 512] psum per 64-row head
                group, write bf16 into dst[:, i*512:(i+1)*512]."""
                sq = sqp.tile([DC, SCHUNK], F32R, name=f"sq_{b}_{i}", tag="sq")
                nc.scalar.activation(sq[:], acc_psum[:],
                                     mybir.ActivationFunctionType.Square)
                sumsq = ps_tiny.tile([2, SCHUNK], F32, name=f"ss_{b}_{i}", tag="tiny")
                nc.tensor.matmul(sumsq[:], sel2_sb[:], sq[:], start=True, stop=True)
                # rstd = (mean + eps)^-0.5 = exp(-0.5 * ln(mean + eps));
                # ln and exp share one ACT table (natural_log_exp_and_others),
                # so this avoids per-chunk ACT table reloads that Sqrt would
                # force.
                lm = stdp.tile([2, SCHUNK], F32, name=f"lm_{b}_{i}", tag="std")
                nc.scalar.activation(lm[:], sumsq[:],
                                     mybir.ActivationFunctionType.Ln,
                                     scale=1.0 / DH, bias=eps_sb[:2, :])
                rstd = stdp.tile([2, SCHUNK], F32R, name=f"rstd_{b}_{i}", tag="rstd")
                nc.scalar.activation(rstd[:], lm[:],
                                     mybir.ActivationFunctionType.Exp,
                                     scale=-0.5)
                bc = ps_tiny.tile([DC, SCHUNK], F32, name=f"bc_{b}_{i}", tag="tiny")
                nc.tensor.matmul(bc[:], sel2t_sb[:], rstd[:], start=True, stop=True)
                bcs = sqp.tile([DC, SCHUNK], F32, name=f"bcs_{b}_{i}", tag="bcs")
                nc.vector.tensor_copy(bcs[:], bc[:])
                nc.vector.tensor_mul(dst[:, i * SCHUNK:(i + 1) * SCHUNK],
                                     acc_psum[:], bcs[:])

            for b in range(B):
                for i in range(NCH):
                    col0 = b * S + i * SCHUNK
                    # ---- load xT chunk ----
                    xch = []
                    for k in range(KT):
                        t = xcp.tile([128, SCHUNK], F32R, name=f"x_{b}_{i}_{k}",
                                     tag="xc")
                        nc.sync.dma_start(t[:], xt_d[k * 128:(k + 1) * 128,
                                                     col0:col0 + SCHUNK])
                        xch.append(t)

                    # ---- projections (fp32r) ----
                    psq = ps_acc.tile([DC, SCHUNK], F32, name=f"pq_{b}_{i}", tag="acc")
                    psk = ps_acc.tile([DC, SCHUNK], F32, name=f"pk_{b}_{i}", tag="acc")
                    psv = ps_acc.tile([DC, SCHUNK], F32, name=f"pv_{b}_{i}", tag="acc")
                    for k in range(KT):
                        nc.tensor.matmul(psq[:], w_sb[("q", k)][:], xch[k][:],
                                         start=(k == 0), stop=(k == KT - 1))
                    for k in range(KT):
                        nc.tensor.matmul(psk[:], w_sb[("k", k)][:], xch[k][:],
                                         start=(k == 0), stop=(k == KT - 1))
                    for k in range(KT):
                        nc.tensor.matmul(psv[:], w_sb[("v", k)][:], xch[k][:],
                                         start=(k == 0), stop=(k == KT - 1))

                    rms_norm_store(psq, qt_sb[b], b, i)
                    rms_norm_store(psk, kt_sb[b], b, i)

                    # ---- v: copy to bf16, transpose into v_aug tiles ----
                    vt = vtp.tile([DC, SCHUNK], BF, name=f"vt_{b}_{i}", tag="vt")
                    nc.vector.tensor_copy(vt[:], psv[:])
                    for u in range(SCHUNK // 128):
                        j = i * (SCHUNK // 128) + u
                        tp = ps_pt.tile([128, 128], BF, name=f"tp_{b}_{j}", tag="pt")
                        nc.tensor.transpose(tp[:], vt[:, u * 128:(u + 1) * 128],
                                            ident_sb[:])
                        va = vaugp.tile([128, 2 * (DH + 1)], BF,
                                        name=f"va_{b}_{j}", tag="vaug")
                        nc.vector.tensor_copy(va[:, 0:DH], tp[:, 0:DH])
                        nc.vector.tensor_copy(va[:, DH + 1:2 * DH + 1],
                                              tp[:, DH:2 * DH])
                        nc.vector.memset(va[:, DH:DH + 1], 1.0)
                        nc.vector.memset(va[:, 2 * DH + 1:2 * DH + 2], 1.0)
                        vaug[(b, j)] = va

                    # ---- attention for (b, i) ----
                    att = [ps_att.tile([DH + 1, SCHUNK], F32,
                                       name=f"att_{b}_{i}_{h}", tag="att")
                           for h in range(HEADS_PER_CORE)]
                    n_t = 4 * i + 4
                    for j in range(n_t):
                        off = max(0, 128 * (j - 4 * i))
                        npx = SCHUNK - off
                        pts = []
                        for h in range(HEADS_PER_CORE):
                            pt = ps_pt.tile([128, SCHUNK], F32,
                                            name=f"ptile_{b}_{i}_{j}_{h}", tag="pt")
                            nc.tensor.matmul(
                                pt[:, :npx],
                                kt_sb[b][h * DH:(h + 1) * DH, j * TT:(j + 1) * TT],
                                qt_sb[b][h * DH:(h + 1) * DH,
                                         i * SCHUNK + off:(i + 1) * SCHUNK],
                                start=True, stop=True,
                                tile_position=(h * DH, 0),
                            )
                            pts.append(pt)
                        for h in range(HEADS_PER_CORE):
                            psb = pp.tile([128, SCHUNK], BF,
                                          name=f"p_{b}_{i}_{j}_{h}", tag="p")
                            nc.scalar.activation(psb[:, :npx], pts[h][:, :npx],
                                                 mybir.ActivationFunctionType.Exp,
                                                 scale=SCALE)
                            if j >= 4 * i:
                                nc.vector.tensor_mul(psb[:, :npx], psb[:, :npx],
                                                     mask_sb[:, :npx])
                            nc.tensor.matmul(
                                att[h][:, off:SCHUNK],
                                vaug[(b, j)][:, h * (DH + 1):(h + 1) * (DH + 1)],
                                psb[:, :npx],
                                start=(j == 0), stop=(j == n_t - 1),
                            )

                    # ---- normalize by softmax denominator ----
                    at_all = attallp.tile([DC, SCHUNK], F32R,
                                          name=f"atall_{b}_{i}", tag="attall")
                    for h in range(HEADS_PER_CORE):
                        rc = rcp.tile([1, SCHUNK], F32R, name=f"rc_{b}_{i}_{h}",
                                      tag="rc")
                        with nc.allow_low_precision(
                                reason="f32r rounding feeds PE broadcast"):
                            nc.vector.reciprocal(rc[:], att[h][DH:DH + 1, :])
                        zb = ps_tiny.tile([DH, SCHUNK], F32,
                                          name=f"zb_{b}_{i}_{h}", tag="tiny")
                        nc.tensor.matmul(zb[:], ones64_sb[:], rc[:],
                                         start=True, stop=True)
                        zbs = zbp.tile([DH, SCHUNK], F32, name=f"zbs_{b}_{i}_{h}",
                                       tag="zb")
                        nc.vector.tensor_copy(zbs[:], zb[:])
                        nc.vector.tensor_mul(at_all[h * DH:(h + 1) * DH, :],
                                             att[h][0:DH, :], zbs[:])

                    # ---- partial output projection (fp32r) ----
                    for u in range(SCHUNK // 128):
                        for n in range(D // 512):
                            op = ps_pt.tile([128, 512], F32,
                                            name=f"op_{b}_{i}_{u}_{n}", tag="pt")
                            nc.tensor.matmul(op[:],
                                             at_all[:, u * 128:(u + 1) * 128],
                                             wo_sb[:, n * 512:(n + 1) * 512],
                                             start=True, stop=True)
                            osb = outsbp.tile([128, 512], F32,
                                              name=f"osb_{b}_{i}_{u}_{n}",
                                              tag="outsb")
                            nc.vector.tensor_copy(osb[:], op[:])
                            r0 = b * S + i * SCHUNK + u * 128
                            nc.sync.dma_start(
                                partial[r0:r0 + 128, n * 512:(n + 1) * 512],
                                osb[:])

            # ---- ReduceScatter partial outputs across the 8 cores ----
            nc.gpsimd.collective_compute(
                "ReduceScatter",
                mybir.AluOpType.add,
                replica_groups=[list(range(N_CORES))],
                ins=[partial[:]],
                outs=[rs_out[:]],
            )
            nc.sync.dma_start(out_d[:], rs_out[:])

    nc.compile()
    return nc


_NC_CACHE = {}


def _get_nc():
    if "nc" not in _NC_CACHE:
        _NC_CACHE["nc"] = build_nc()
    return _NC_CACHE["nc"]


def _host_inputs(x, Wq, Wk, Wv, Wo):
    xt = np.ascontiguousarray(x.reshape(ROWS, D).T).astype(np.float32)
    mask0 = (np.arange(TT)[:, None] <= np.arange(SCHUNK)[None, :]).astype(BF16)
    ident = np.eye(128, dtype=BF16)
    sel2 = np.zeros((128, 2), dtype=np.float32)
    sel2[:DH, 0] = 1.0
    sel2[DH:2 * DH, 1] = 1.0
    sel2t = np.ascontiguousarray(sel2.T)
    ones64 = np.ones((1, DH), dtype=np.float32)

    in_maps = []
    for c in range(N_CORES):
        cs = c * DC
        in_maps.append({
            "xt": xt,
            "wq": np.ascontiguousarray(Wq[:, cs:cs + DC]).astype(np.float32),
            "wk": np.ascontiguousarray(Wk[:, cs:cs + DC]).astype(np.float32),
            "wv": np.ascontiguousarray(Wv[:, cs:cs + DC]).astype(np.float32),
            "wo": np.ascontiguousarray(Wo[cs:cs + DC, :]).astype(np.float32),
            "mask0": mask0,
            "ident": ident,
            "sel2": sel2,
            "sel2t": sel2t,
            "ones64": ones64,
        })
    return in_maps


def kernel(x, Wq, Wk, Wv, Wo, mask):
    x = np.asarray(x, dtype=np.float32)
    nc = _get_nc()
    in_maps = _host_inputs(x, np.asarray(Wq), np.asarray(Wk),
                           np.asarray(Wv), np.asarray(Wo))
    res = run_bass_kernel_spmd(nc, in_maps, list(range(N_CORES)))
    full = np.concatenate([res.results[c]["out"] for c in range(N_CORES)], axis=0)
    return full.reshape(B, S, D)


if __name__ == "__main__":
    nc = build_nc()
    print("kernel built and compiled OK")
